# revision 2
# baseline (speedup 1.0000x reference)
"""Bilinear grid-sample for Trainium2 (Bass/Tile), batch-parallel on 8 cores.

im:   [8, 512, 512, 16] f32 NHWC      grid: [8, 2, 512, 512] f32 in [-1, 1]
out:  [8, 512, 512, 16] f32

The wall clock is dominated by host<->device transfer over the axon tunnel
(~50 MB/s), so the kernel minimizes bytes on the wire:
  im   -> int8, scale 32        (32 MB up;   rel-err contribution ~1e-2)
  grid -> int16, 1/16-px fix    ( 8 MB up;   ~2.7e-4)
  out  -> int8 + per-pixel fp16 scale (36 MB down; ~6e-3). The 16 channels
          of a pixel share bilinear weights, so per-pixel max|out|/127 is a
          tight scale. A plain fp16 output would overflow: the reference
          extrapolates out-of-range coords (|out| up to ~2e6), and those
          outliers dominate the Frobenius norm of the expected output.
and avoids uploading donated output buffers by chaining: each call donates
the previous call's device-resident output buffer (the kernel overwrites
every output element, so stale content is harmless). Uploads/downloads are
issued per-core shard, async, so core 0 computes while later shards stream.

Per-core device program (fp16 data path, f32 coordinates/weights):
  1. Full-patch scratch in DRAM: entry(y, x) = 64 fp16
     [im[y,x], im[y,x+1], im[y+1,x], im[y+1,x+1]] decoded from int8 via
     shifted on-chip tensor_scalar copies (mult 1/32).
  2. x0/y0/wx1/wy1 and idx = y0*512 + x0 on DVE (f32), grid decoded
     from int16 via mult 1/16.
  3. Gather one 128B patch per output pixel with [P,1]-offset
     indirect_dma_start (128 pixels per instruction).
  4. Upcast patches to f32, bilinear blend with per-(partition, column)
     weights broadcast over channels, then per-pixel absmax-reduce ->
     reciprocal -> scale to int8 (MAGIC round), store q int8 + scale fp16.
"""

import sys
from concurrent.futures import ThreadPoolExecutor

import numpy as np

sys.path.insert(0, "/opt/trn_rl_repo")

import jax
from jax.experimental.shard_map import shard_map
from jax.sharding import Mesh, NamedSharding, PartitionSpec

from concourse import bacc, mybir, tile
from concourse.bass import IndirectOffsetOnAxis
from concourse.bass2jax import (
    _bass_exec_p,
    install_neuronx_cc_hook,
    partition_id_tensor,
)

F32 = mybir.dt.float32
F16 = mybir.dt.float16
BF16 = mybir.dt.bfloat16
I8 = mybir.dt.int8
I16 = mybir.dt.int16
I32 = mybir.dt.int32
ALU = mybir.AluOpType

B = 8
H = W = 512
C = 16
P = 128
NPP = (H * W) // P  # 2048 pixel-columns per partition-row
GB = 128  # gather columns per blend batch
NB = NPP // GB  # 16 blend batches
MAGIC = 8388608.0  # 2^23: (x + MAGIC) - MAGIC rounds fp32 to nearest integer
IM_SCALE = 32.0  # im int8 quantization scale
GRID_SCALE = 16.0  # grid int16 fixed-point: units of 1/16 pixel

_POOL = ThreadPoolExecutor(16)


def _build_scratch(nc, sc_d, im_d, tc):
    """scratch[y*512+x] = [im[y,x], im[y,x+1], im[y+1,x], im[y+1,x+1]] (64 fp16),
    decoded from int8 im with mult 1/IM_SCALE."""

    def dec(out_ap, in_ap):
        nc.vector.tensor_scalar(
            out=out_ap, in0=in_ap, scalar1=1.0 / IM_SCALE, scalar2=None,
            op0=ALU.mult, op1=ALU.bypass,
        )

    with tc.tile_pool(name="bld", bufs=1) as bp:
        # batches of 127 output rows from 128 loaded rows
        starts = [0, 127, 254, 381]
        for r in starts:
            a = bp.tile([127, W * C], I8, tag="a")
            nc.sync.dma_start(
                out=a[:], in_=im_d[r : r + 127, :, :].rearrange("r x c -> r (x c)")
            )
            a1 = bp.tile([127, W * C], I8, tag="a1")
            nc.sync.dma_start(
                out=a1[:], in_=im_d[r + 1 : r + 128, :, :].rearrange("r x c -> r (x c)")
            )
            for h in range(2):
                s = bp.tile([127, 256 * 64], F16, tag="s")
                sv = s[:].rearrange("p (x e) -> p x e", e=64)
                xo = 256 * h * C
                # corner (y, x)
                dec(
                    sv[:, :, 0:16],
                    a[0:127, xo : xo + 4096].rearrange("p (x c) -> p x c", c=16),
                )
                # corner (y, x+1); at x=511 the source would be off the end -> skip last col
                nx = 256 if h == 0 else 255
                if nx == 255:
                    nc.vector.memset(sv[:, 255:256, 16:32], 0.0)
                    nc.vector.memset(sv[:, 255:256, 48:64], 0.0)
                dec(
                    sv[:, 0:nx, 16:32],
                    a[0:127, xo + 16 : xo + 16 + nx * 16].rearrange(
                        "p (x c) -> p x c", c=16
                    ),
                )
                # corner (y+1, x)
                dec(
                    sv[:, :, 32:48],
                    a1[0:127, xo : xo + 4096].rearrange("p (x c) -> p x c", c=16),
                )
                # corner (y+1, x+1)
                dec(
                    sv[:, 0:nx, 48:64],
                    a1[0:127, xo + 16 : xo + 16 + nx * 16].rearrange(
                        "p (x c) -> p x c", c=16
                    ),
                )
                nc.sync.dma_start(
                    out=sc_d[r : r + 127, h * 256 : (h + 1) * 256, :].rearrange(
                        "y x e -> y (x e)"
                    ),
                    in_=s[:],
                )
        # tail rows 508..510 (3 entry rows, uses im rows 508..511)
        a = bp.tile([127, W * C], I8, tag="a")
        nc.sync.dma_start(
            out=a[0:3, :], in_=im_d[508:511, :, :].rearrange("r x c -> r (x c)")
        )
        a1 = bp.tile([127, W * C], I8, tag="a1")
        nc.sync.dma_start(
            out=a1[0:3, :], in_=im_d[509:512, :, :].rearrange("r x c -> r (x c)")
        )
        for h in range(2):
            s = bp.tile([127, 256 * 64], F16, tag="s")
            sv = s[:].rearrange("p (x e) -> p x e", e=64)
            xo = 256 * h * C
            nx = 256 if h == 0 else 255
            if nx == 255:
                nc.vector.memset(sv[0:3, 255:256, 16:32], 0.0)
                nc.vector.memset(sv[0:3, 255:256, 48:64], 0.0)
            dec(
                sv[0:3, :, 0:16],
                a[0:3, xo : xo + 4096].rearrange("p (x c) -> p x c", c=16),
            )
            dec(
                sv[0:3, 0:nx, 16:32],
                a[0:3, xo + 16 : xo + 16 + nx * 16].rearrange(
                    "p (x c) -> p x c", c=16
                ),
            )
            dec(
                sv[0:3, :, 32:48],
                a1[0:3, xo : xo + 4096].rearrange("p (x c) -> p x c", c=16),
            )
            dec(
                sv[0:3, 0:nx, 48:64],
                a1[0:3, xo + 16 : xo + 16 + nx * 16].rearrange(
                    "p (x c) -> p x c", c=16
                ),
            )
            nc.sync.dma_start(
                out=sc_d[508:511, h * 256 : (h + 1) * 256, :].rearrange(
                    "y x e -> y (x e)"
                ),
                in_=s[0:3, :],
            )


def _build_program():
    nc = bacc.Bacc(
        "TRN2", target_bir_lowering=False, debug=False, enable_asserts=False
    )

    im_d = nc.dram_tensor("im", [H, W, C], I8, kind="ExternalInput")
    gx_d = nc.dram_tensor("gx", [P, NPP], I16, kind="ExternalInput")
    gy_d = nc.dram_tensor("gy", [P, NPP], I16, kind="ExternalInput")
    out_d = nc.dram_tensor("out", [P, NPP * C], I8, kind="ExternalOutput")
    os_d = nc.dram_tensor("osc", [P, NPP], F16, kind="ExternalOutput")
    sc_d = nc.dram_tensor("scratch", [H, W, 64], F16)

    with tile.TileContext(nc) as tc:
        _build_scratch(nc, sc_d, im_d, tc)

        with tc.tile_pool(name="persist", bufs=1) as pp:
            wx1 = pp.tile([P, NPP], F32, tag="wx1")
            wy1 = pp.tile([P, NPP], F32, tag="wy1")
            idx_i = pp.tile([P, NPP], I32, tag="idx")

            with tc.tile_pool(name="scratchp", bufs=1) as sp:

                def axis_setup(src_dram, x0_tag, w1_out):
                    raw = sp.tile([P, NPP], I16, tag="sraw")
                    nc.sync.dma_start(out=raw[:], in_=src_dram[:])
                    # g = raw / GRID_SCALE: pixel-space coordinate, f32
                    g = sp.tile([P, NPP], F32, tag="s2")
                    nc.vector.tensor_scalar(
                        out=g[:], in0=raw[:], scalar1=1.0 / GRID_SCALE, scalar2=None,
                        op0=ALU.mult, op1=ALU.bypass,
                    )
                    t = sp.tile([P, NPP], F32, tag="s3")
                    nc.vector.tensor_scalar(
                        out=t[:], in0=g[:], scalar1=0.0, scalar2=510.5,
                        op0=ALU.max, op1=ALU.min,
                    )
                    r = sp.tile([P, NPP], F32, tag="s1")
                    nc.vector.tensor_scalar(
                        out=r[:], in0=t[:], scalar1=MAGIC, scalar2=MAGIC,
                        op0=ALU.add, op1=ALU.subtract,
                    )
                    d = sp.tile([P, NPP], F32, tag="s4")
                    nc.vector.tensor_tensor(out=d[:], in0=r[:], in1=t[:], op=ALU.is_gt)
                    x0 = sp.tile([P, NPP], F32, tag=x0_tag)
                    nc.vector.tensor_tensor(
                        out=x0[:], in0=r[:], in1=d[:], op=ALU.subtract
                    )
                    nc.vector.tensor_tensor(
                        out=w1_out[:], in0=g[:], in1=x0[:], op=ALU.subtract
                    )
                    return x0

                x0f = axis_setup(gx_d, "x0x", wx1)
                y0f = axis_setup(gy_d, "x0y", wy1)

                idxf = sp.tile([P, NPP], F32, tag="s1")
                nc.vector.scalar_tensor_tensor(
                    out=idxf[:], in0=y0f[:], scalar=float(W), in1=x0f[:],
                    op0=ALU.mult, op1=ALU.add,
                )
                nc.vector.tensor_copy(out=idx_i[:], in_=idxf[:])

            with (
                tc.tile_pool(name="gather", bufs=2) as gp,
                tc.tile_pool(name="conv", bufs=2) as cp,
                tc.tile_pool(name="work", bufs=2) as wp,
                tc.tile_pool(name="wts", bufs=2) as wtp,
            ):
                for b in range(NB):
                    tb = gp.tile([P, GB, 64], F16, tag="tb")
                    for gi in range(GB):
                        n = b * GB + gi
                        nc.gpsimd.indirect_dma_start(
                            out=tb[:, gi, :],
                            out_offset=None,
                            in_=sc_d[:],
                            in_offset=IndirectOffsetOnAxis(
                                ap=idx_i[:, n : n + 1], axis=1
                            ),
                            element_offset=0,
                        )

                    tf = cp.tile([P, GB, 64], F32, tag="tf")
                    nc.vector.tensor_copy(out=tf[:], in_=tb[:])

                    sl = slice(b * GB, (b + 1) * GB)
                    m = wtp.tile([P, GB, 1], F32, tag="m")
                    nc.vector.tensor_tensor(
                        out=m[:, :, 0], in0=wx1[:, sl], in1=wy1[:, sl], op=ALU.mult
                    )
                    w10 = wtp.tile([P, GB, 1], F32, tag="w10")
                    nc.vector.tensor_tensor(
                        out=w10[:, :, 0], in0=wx1[:, sl], in1=m[:, :, 0],
                        op=ALU.subtract,
                    )
                    w01 = wtp.tile([P, GB, 1], F32, tag="w01")
                    nc.vector.tensor_tensor(
                        out=w01[:, :, 0], in0=wy1[:, sl], in1=m[:, :, 0],
                        op=ALU.subtract,
                    )
                    u = wtp.tile([P, GB, 1], F32, tag="u")
                    nc.vector.tensor_tensor(
                        out=u[:, :, 0], in0=m[:, :, 0], in1=wx1[:, sl], op=ALU.subtract
                    )
                    w00 = wtp.tile([P, GB, 1], F32, tag="w00")
                    nc.vector.scalar_tensor_tensor(
                        out=w00[:, :, 0], in0=u[:, :, 0], scalar=1.0, in1=wy1[:, sl],
                        op0=ALU.add, op1=ALU.subtract,
                    )

                    shp = [P, GB, C]
                    a = wp.tile(shp, F32, tag="a")
                    bb = wp.tile(shp, F32, tag="b")
                    nc.vector.tensor_tensor(
                        out=a[:], in0=tf[:, :, 0:16], in1=w00[:].to_broadcast(shp),
                        op=ALU.mult,
                    )
                    nc.vector.tensor_tensor(
                        out=bb[:], in0=tf[:, :, 16:32], in1=w10[:].to_broadcast(shp),
                        op=ALU.mult,
                    )
                    nc.vector.tensor_tensor(out=a[:], in0=a[:], in1=bb[:], op=ALU.add)
                    nc.vector.tensor_tensor(
                        out=bb[:], in0=tf[:, :, 32:48], in1=w01[:].to_broadcast(shp),
                        op=ALU.mult,
                    )
                    nc.vector.tensor_tensor(out=a[:], in0=a[:], in1=bb[:], op=ALU.add)
                    nc.vector.tensor_tensor(
                        out=bb[:], in0=tf[:, :, 48:64], in1=m[:].to_broadcast(shp),
                        op=ALU.mult,
                    )
                    o = wp.tile(shp, F32, tag="o")
                    nc.vector.tensor_tensor(out=o[:], in0=a[:], in1=bb[:], op=ALU.add)

                    # per-pixel int8 quantization: s = max|o| over 16 channels
                    s = wtp.tile([P, GB], F32, tag="s")
                    nc.vector.tensor_reduce(
                        out=s[:], in_=o[:], axis=mybir.AxisListType.X,
                        op=ALU.max, apply_absolute_value=True,
                    )
                    # sf = max(s, tiny) / 127  (shipped scale: out = q * sf).
                    # reciprocal needs f32 input (HW DVE recip assumes f32 bit
                    # layout); ship a separate fp16 copy (max|out|/127 << 65504)
                    sf = wtp.tile([P, GB], F32, tag="sf")
                    nc.vector.tensor_scalar(
                        out=sf[:], in0=s[:], scalar1=1e-30, scalar2=1.0 / 127.0,
                        op0=ALU.max, op1=ALU.mult,
                    )
                    rec = wtp.tile([P, GB, 1], F32, tag="rec")
                    nc.vector.reciprocal(out=rec[:, :, 0], in_=sf[:])
                    sf16 = wtp.tile([P, GB], F16, tag="sf16")
                    nc.vector.tensor_copy(out=sf16[:], in_=sf[:])
                    q = wp.tile(shp, F32, tag="q")
                    nc.vector.tensor_tensor(
                        out=q[:], in0=o[:], in1=rec[:].to_broadcast(shp), op=ALU.mult
                    )
                    nc.vector.tensor_scalar(
                        out=q[:], in0=q[:], scalar1=MAGIC, scalar2=MAGIC,
                        op0=ALU.add, op1=ALU.subtract,
                    )
                    qi = wp.tile(shp, I8, tag="qi")
                    nc.vector.tensor_copy(out=qi[:], in_=q[:])

                    nc.sync.dma_start(
                        out=out_d[:, b * GB * C : (b + 1) * GB * C],
                        in_=qi[:, :, :],
                    )
                    nc.sync.dma_start(
                        out=os_d[:, b * GB : (b + 1) * GB], in_=sf16[:]
                    )

    nc.compile()
    return nc


def _enc_im(im_b):
    """[512,512,16] f32 -> int8 at scale IM_SCALE (round-to-nearest, clip)."""
    t = im_b * IM_SCALE
    np.rint(t, out=t)
    np.clip(t, -127.0, 127.0, out=t)
    return t.astype(np.int8)


def _enc_grid(plane):
    """[512,512] f32 in [-1,1] -> [P,NPP] int16 pixel-coords * GRID_SCALE."""
    t = (plane + 1.0) * (256.0 * GRID_SCALE)
    np.rint(t, out=t)
    np.clip(t, -32768.0, 32767.0, out=t)
    return t.astype(np.int16).reshape(P, NPP)


def make_feeds(im_b, grid_b):
    """Per-core feed dict from one batch's f32 inputs (for CoreSim / tests)."""
    return {
        "im": _enc_im(np.asarray(im_b, np.float32).copy()),
        "gx": _enc_grid(np.asarray(grid_b[0], np.float32)),
        "gy": _enc_grid(np.asarray(grid_b[1], np.float32)),
    }


def decode_out(q_raw, s_raw):
    """int8 [P, NPP*C] + f32 scales [P, NPP] -> [H, W, C] f32."""
    q = np.asarray(q_raw).reshape(P, NPP, C).astype(np.float32)
    s = np.asarray(s_raw).reshape(P, NPP)
    return (q * s[:, :, None]).reshape(H, W, C)


def sim_outputs(sim):
    return decode_out(np.asarray(sim.tensor("out")), np.asarray(sim.tensor("osc")))


class _Res:
    exec_time_ns = None
    mean_exec_time_ns = None
    results = None


_STATE = None


def _get_nc():
    return _get_state()["nc"]


def _get_state():
    global _STATE
    if _STATE is None:
        nc = _build_program()
        install_neuronx_cc_hook()

        partition_name = (
            nc.partition_id_tensor.name if nc.partition_id_tensor else None
        )
        in_names, out_names, out_avals, zero_outs = [], [], [], []
        for alloc in nc.m.functions[0].allocations:
            if not isinstance(alloc, mybir.MemoryLocationSet):
                continue
            name = alloc.memorylocations[0].name
            if alloc.kind == "ExternalInput":
                if name != partition_name:
                    in_names.append(name)
            elif alloc.kind == "ExternalOutput":
                shape = tuple(alloc.tensor_shape)
                dtype = mybir.dt.np(alloc.dtype)
                out_avals.append(jax.core.ShapedArray(shape, dtype))
                zero_outs.append(np.zeros((B * shape[0], *shape[1:]), dtype))
                out_names.append(name)
        n_params = len(in_names)
        n_outs = len(out_avals)
        all_in_names = list(in_names) + list(out_names)
        if partition_name is not None:
            all_in_names.append(partition_name)

        def _body(*args):
            operands = list(args)
            if partition_name is not None:
                operands.append(partition_id_tensor())
            outs = _bass_exec_p.bind(
                *operands,
                out_avals=tuple(out_avals),
                in_names=tuple(all_in_names),
                out_names=tuple(out_names),
                lowering_input_output_aliases=(),
                sim_require_finite=True,
                sim_require_nnan=True,
                nc=nc,
            )
            return tuple(outs)

        devices = jax.devices()[:B]
        mesh = Mesh(np.asarray(devices), ("core",))
        sh = NamedSharding(mesh, PartitionSpec("core"))
        in_specs = (PartitionSpec("core"),) * (n_params + n_outs)
        out_specs = (PartitionSpec("core"),) * n_outs
        sharded = jax.jit(
            shard_map(
                _body, mesh=mesh, in_specs=in_specs, out_specs=out_specs,
                check_rep=False,
            ),
            donate_argnums=tuple(range(n_params, n_params + n_outs)),
            keep_unused=True,
        )
        _STATE = {
            "nc": nc,
            "sharded": sharded,
            "sh": sh,
            "devices": devices,
            "in_names": in_names,
            "out_names": out_names,
            # device-resident donation buffers for the next call (the kernel
            # fully overwrites out, so content is irrelevant)
            "donate": [jax.device_put(z, sh) for z in zero_outs],
        }
    return _STATE


_GLOBAL_SHAPES = {
    "im": (B * H, W, C),
    "gx": (B * P, NPP),
    "gy": (B * P, NPP),
}


def _run(im, grid, trace=False):
    st = _get_state()
    devices, sh = st["devices"], st["sh"]
    im = np.asarray(im)
    grid = np.asarray(grid)

    # encode per-core shards on the thread pool, then issue async uploads
    # in core order so core 0's compute starts while later shards stream
    def enc_core(b):
        return {
            "im": _enc_im(im[b]),
            "gx": _enc_grid(grid[b, 0]),
            "gy": _enc_grid(grid[b, 1]),
        }

    shards = {n: [] for n in _GLOBAL_SHAPES}
    for b in range(B):
        e = enc_core(b)
        for n in ("im", "gx", "gy"):
            shards[n].append(jax.device_put(e[n], devices[b]))
    gargs = {
        n: jax.make_array_from_single_device_arrays(
            _GLOBAL_SHAPES[n], sh, shards[n]
        )
        for n in _GLOBAL_SHAPES
    }

    args = [gargs[n] for n in st["in_names"]]
    outs = st["sharded"](*args, *st["donate"])
    st["donate"] = list(outs)

    oq = outs[st["out_names"].index("out")]
    osc = outs[st["out_names"].index("osc")]
    for o in (oq, osc):
        for s in o.addressable_shards:
            try:
                s.data.copy_to_host_async()
            except Exception:
                pass
    sc_shards = {s.index[0].start // P: s.data for s in osc.addressable_shards}
    final = np.empty((B, H, W, C), np.float32)

    def fetch(s):
        b = s.index[0].start // P
        q = np.asarray(s.data).reshape(P, NPP, C).astype(np.float32)
        sc = np.asarray(sc_shards[b]).reshape(P, NPP)
        np.multiply(q, sc[:, :, None], out=final[b].reshape(P, NPP, C))

    list(_POOL.map(fetch, oq.addressable_shards))
    return final, _Res()


def kernel(im, grid):
    out, _ = _run(np.asarray(im), np.asarray(grid))
    return out
